# revision 1
# baseline (speedup 1.0000x reference)
"""Trainium2 Bass kernel for nn_DilateMask (16x1x1024x1024 f32 mask, 5 iterations).

The reference iterates 5x: edge-pad, 3x3 discrete-Laplacian conv, then
`mask[|res| > 1e-4] = 1`. For a binary mask each iteration is exactly binary
dilation by the 3x3 cross (out-of-bounds neighbors never contribute: the edge
padding replicates the center pixel, which is 0 in the only case that
matters). Five iterations == dilation by the L1 diamond of radius 5.

Kernel (pure data parallel, 2 images per NeuronCore x 8 cores; per 128-row
chunk, chunks overlap 5 rows each side so every chunk is independent):
  1. DMA mask rows into SBUF (HWDGE on the SP sequencer; only input DMAs
     live there so its queue never stalls on late pipeline stages).
  2. Horizontal decay field on DVE via three log-doubling levels
     (shifts 1, 2, 4 on a zero-padded bf16 tile, B = 1/32):
       v_k = max(v_{k-1}, B^s * max(v_{k-1}(j-s), v_{k-1}(j+s)))
     giving v(j) = B**dist(j) exactly for horizontal distance dist <= 7
     (binary path decomposition; all values are exact powers of two in
     bf16, and none go subnormal).
  3. One banded matmul per 512-column block on the tensor engine with
     Toeplitz weights B**|di| over the 11-row band:
       T(i,j) = sum_{|di|<=5} B**|di| * v(i+di, j)
     A set pixel inside the diamond implies some term >= B**5; none set
     implies every term <= B**6, so T <= 11*B**6 = 0.34*B**5 -- an exact
     separation (positive-term fp sums stay >= their largest term, and the
     small terms cannot accumulate past the 0.6*B**5 threshold).
  4. Threshold on ACT: sign(T - 0.6*B**5) in {-1,+1}, then the affine map
     0.5*x + 0.5 -> exactly {0.0, 1.0} float32.
  5. Output DMA via SWDGE (gpsimd) so output waits never block input issue.

Built on bacc.Bacc: this toolchain's instruction encodings hold only one
sync-wait each, and Bacc legalizes Tile's multi-wait instructions.
"""
import numpy as np
import ml_dtypes
from contextlib import ExitStack

import concourse.bacc as bacc
import concourse.mybir as mybir
import concourse.tile as tile

B, C, H, W = 16, 1, 1024, 1024
N_CORES = 8
PER_CORE = B // N_CORES
CO = 118               # output rows per chunk (118 + 2*5 halo = 128 input rows)
R = 5                  # dilation radius
BETA = 1.0 / 32.0
TAU = 0.6 * BETA**R
PAD = 8                # zero-pad columns, >= largest doubling shift (4) * 2
NBLK = 512             # PSUM bank width in f32


def _band_matrix(K, M, off):
    """lhsT[k, m] = BETA**|k - m - off| within the 11-row band, else 0."""
    k = np.arange(K)[:, None]
    m = np.arange(M)[None, :]
    d = np.abs(k - m - off)
    w = np.where(d <= R, float(BETA) ** d, 0.0)
    return np.ascontiguousarray(w.astype(ml_dtypes.bfloat16))


def _build(nc, n_img, loop_niter=None):
    """loop_niter: wrap the body in a device-side For_i (timing only)."""
    WP = W + 2 * PAD
    x = nc.dram_tensor("x", [n_img, H, W], mybir.dt.float32, kind="ExternalInput")
    bt = nc.dram_tensor("band_top", [CO + R, CO], mybir.dt.bfloat16,
                        kind="ExternalInput")
    bm = nc.dram_tensor("band_mid", [CO + 2 * R, CO], mybir.dt.bfloat16,
                        kind="ExternalInput")
    y = nc.dram_tensor("y", [n_img, H, W], mybir.dt.float32, kind="ExternalOutput")
    n_chunks = (H + CO - 1) // CO

    with tile.TileContext(nc) as tc, ExitStack() as ctx:
        wpool = ctx.enter_context(tc.tile_pool(name="weights", bufs=1))
        mpool = ctx.enter_context(tc.tile_pool(name="m", bufs=4))
        vpool = ctx.enter_context(tc.tile_pool(name="v", bufs=3))
        opool = ctx.enter_context(tc.tile_pool(name="out", bufs=4))
        ppool = ctx.enter_context(tc.tile_pool(name="psum", bufs=4, space="PSUM"))
        cpool = ctx.enter_context(tc.tile_pool(name="const", bufs=1))

        band_top_t = wpool.tile([CO + R, CO], mybir.dt.bfloat16)
        nc.sync.dma_start(band_top_t[:], bt[:])
        band_mid_t = wpool.tile([CO + 2 * R, CO], mybir.dt.bfloat16)
        nc.sync.dma_start(band_mid_t[:], bm[:])
        ntau_t = cpool.tile([128, 1], mybir.dt.float32)
        nc.vector.memset(ntau_t[:], -TAU)
        half_t = cpool.tile([128, 1], mybir.dt.float32)
        nc.vector.memset(half_t[:], 0.5)

        loop = tc.For_i(0, loop_niter, 1) if loop_niter else None
        if loop:
            loop.__enter__()
        for img in range(n_img):
            for c in range(n_chunks):
                o0 = c * CO
                o1 = min(o0 + CO, H)
                i0 = max(o0 - R, 0)
                i1 = min(o1 + R, H)
                K = i1 - i0
                M = o1 - o0
                lhsT = band_top_t[:K, :M] if c == 0 else band_mid_t[:K, :M]

                mt = mpool.tile([128, W], mybir.dt.float32, tag="m")
                nc.sync.dma_start(mt[:K, :], x[img, i0:i1, :])

                # doubling chain v0 -> v3 (ping-pong, zero-padded columns)
                v0t = vpool.tile([128, WP], mybir.dt.bfloat16, tag="v0")
                v1t = vpool.tile([128, WP], mybir.dt.bfloat16, tag="v1")
                v2t = vpool.tile([128, WP], mybir.dt.bfloat16, tag="v2")
                v3t = vpool.tile([128, WP], mybir.dt.bfloat16, tag="v3")
                vs = [v0t, v1t, v2t, v3t]
                for v in vs:
                    nc.vector.memset(v[:K, :PAD], 0.0)
                    nc.vector.memset(v[:K, WP - PAD:], 0.0)
                nc.scalar.copy(vs[0][:K, PAD:PAD + W], mt[:K, :])  # f32 -> bf16
                for k in range(1, 4):
                    s = 1 << (k - 1)
                    pv, cv = vs[k - 1], vs[k]
                    nc.vector.tensor_tensor(
                        out=cv[:K, PAD:PAD + W],
                        in0=pv[:K, PAD - s:PAD - s + W],
                        in1=pv[:K, PAD + s:PAD + s + W],
                        op=mybir.AluOpType.max)
                    nc.vector.scalar_tensor_tensor(
                        out=cv[:K, PAD:PAD + W], in0=cv[:K, PAD:PAD + W],
                        scalar=float(BETA) ** s, in1=pv[:K, PAD:PAD + W],
                        op0=mybir.AluOpType.mult, op1=mybir.AluOpType.max)
                v3 = vs[3]

                ot = opool.tile([128, W], mybir.dt.float32, tag="out")
                for b in range(W // NBLK):
                    ps = ppool.tile([128, NBLK], mybir.dt.float32, tag="ps")
                    sl = slice(PAD + b * NBLK, PAD + (b + 1) * NBLK)
                    sl_o = slice(b * NBLK, (b + 1) * NBLK)
                    nc.tensor.matmul(ps[:M, :], lhsT, v3[:K, sl],
                                     start=True, stop=True)
                    sg = opool.tile([128, NBLK], mybir.dt.bfloat16, tag="sgn")
                    nc.scalar.activation(sg[:M, :], ps[:M, :],
                                         mybir.ActivationFunctionType.Sign,
                                         bias=ntau_t[:M])
                    nc.scalar.activation(ot[:M, sl_o], sg[:M, :],
                                         mybir.ActivationFunctionType.Identity,
                                         bias=half_t[:M], scale=0.5)
                nc.gpsimd.dma_start(y[img, o0:o1, :], ot[:M, :])
        if loop:
            loop.__exit__(None, None, None)
    return nc


_CACHE = {}


def _get_nc():
    if "nc" not in _CACHE:
        nc = bacc.Bacc("TRN2", target_bir_lowering=False)
        _build(nc, PER_CORE)
        nc.compile()
        _CACHE["nc"] = nc
        _CACHE["bt"] = _band_matrix(CO + R, CO, 0)
        _CACHE["bm"] = _band_matrix(CO + 2 * R, CO, R)
    return _CACHE["nc"], _CACHE["bt"], _CACHE["bm"]


def kernel(batch_mask, weight=None, iter_num=None, **_unused):
    from concourse.bass_utils import run_bass_kernel_spmd

    nc, bt_np, bm_np = _get_nc()
    bm4 = np.ascontiguousarray(np.asarray(batch_mask, dtype=np.float32))
    assert bm4.shape == (B, C, H, W), bm4.shape
    in_maps = []
    for cidx in range(N_CORES):
        xs = np.ascontiguousarray(bm4[cidx * PER_CORE:(cidx + 1) * PER_CORE, 0])
        in_maps.append({"x": xs, "band_top": bt_np, "band_mid": bm_np})
    res = run_bass_kernel_spmd(nc, in_maps, list(range(N_CORES)))
    out = np.concatenate([np.asarray(res.results[cidx]["y"])
                          for cidx in range(N_CORES)], axis=0)
    return out.reshape(B, C, H, W).astype(np.float32)

